# revision 42
# baseline (speedup 1.0000x reference)
"""Trainium2 Bass kernel for nn_AttentionBlock (B=8, LN=2048, IDM=HDM=ODM=1024).

Sharding: data-parallel over batch, one batch element per NeuronCore (8 cores).

Algebra: scores = (i@q)(i@k)^T = i @ W @ i^T with W = q@k^T precomputed on
host in fp32. Per-core computation (batch element b):
    u       = i @ W                 [ln, idm]   (fp32r matmul)
    scores  = u @ i^T               [ln, ln]    (fp32r, u split hi/lo 2-pass)
    att     = softmax(scores, -1)
    vls     = i @ v                 [ln, idm]   (fp32r matmul)
    ret     = att @ vls + i                     (bf16 matmul)
    out     = leaky_relu(ret @ mlp, 0.2) + bias (bf16 matmul)

Precision: the softmax exponent amplifies matmul operand rounding. float32r
runs at full bf16 rate and rounds operands to ~12 mantissa bits (round to
nearest, 11 explicit bits) at SBUF-write / PE-read time. The u operand is
split on-device into an on-grid r11 hi part plus an fp32r lo residual, making
the scores matmul effectively fp32-grade in u and r11-grade in i (measured
max rel err 1.29e-2 vs 2e-2 budget). The value/MLP path tolerates bf16.

Layout: contraction dim on partitions everywhere. iT (= i_b.T) is resident
fp32r in SBUF; u^T hi/lo are staged through DRAM per 512-row s-group; att is
transposed on-chip by DVE 32x32 block transposes.

NOTE: att is transposed with DVE 32x32 block transposes. The xbar DMA
transpose (dma_start_transpose) intermittently corrupts block-max attention
entries whenever other DMA traffic is in flight (known HW issue; this tile
framework does not serialize xbar-mode transitions) -- do not use it here.
"""
import numpy as np
import ml_dtypes

import concourse.bacc as bacc
import concourse.mybir as mybir
import concourse.tile as tile
from concourse import bass_utils

F32 = mybir.dt.float32
F32R = mybir.dt.float32r
BF16 = mybir.dt.bfloat16
I32 = mybir.dt.int32
Act = mybir.ActivationFunctionType
Axis = mybir.AxisListType

LN = 2048      # sequence length
D = 1024       # idm = hdm = odm
N_CORES = 8
DC = D // 128      # 8 contraction chunks
ST = LN // 128     # 16 s-tiles
TB = LN // 512     # 4 t-blocks (N=512)
G = LN // 512      # 4 s-groups
NEG_SLOPE = 0.2

_cached_nc = None


def _build(dbg=0):
    nc = bacc.Bacc("TRN2", target_bir_lowering=False, debug=False)

    iT = nc.dram_tensor("iT", [D, LN], F32R, kind="ExternalInput")
    w = nc.dram_tensor("w", [D, D], F32R, kind="ExternalInput")      # q @ k.T
    v = nc.dram_tensor("v", [D, D], F32R, kind="ExternalInput")
    mlpb = nc.dram_tensor("mlpb", [D, D], BF16, kind="ExternalInput")
    bias = nc.dram_tensor("bias", [LN, D], BF16, kind="ExternalInput")
    out_d = nc.dram_tensor("out", [LN, D], F32, kind="ExternalOutput")
    if dbg:
        attd = nc.dram_tensor("attd", [ST, 128, LN], BF16, kind="ExternalOutput")
        attTd = nc.dram_tensor("attTd", [G, 128, ST, 512], BF16,
                               kind="ExternalOutput")

    # [D, X] viewed as [128 partitions, DC chunks, X]
    def pcv(t, x):
        return t.ap().rearrange("(c p) x -> p c x", p=128)

    with tile.TileContext(nc) as tc:
        with tc.tile_pool(name="pers", bufs=1) as pers, \
             tc.tile_pool(name="dram", bufs=1, space="DRAM") as dram:
            iT_sb = pers.tile([128, DC, LN], F32R)     # 64 KB/part, resident
            vls_sb = pers.tile([128, ST, D], BF16)     # 32 KB/part, resident
            mlp_sb = pers.tile([128, DC, D], BF16)     # 16 KB/part, resident
            alpha_ap = pers.tile([128, 1], F32)
            nc.vector.memset(alpha_ap, NEG_SLOPE)

            uT_d = dram.tile([G, 128, DC, 512], F32R)   # staged u^T hi (r11)
            uTl_d = dram.tile([G, 128, DC, 512], F32R)  # staged u^T lo

            _psum_cm = tc.tile_pool(name="psum", bufs=1, space="PSUM")
            psum_pool = _psum_cm.__enter__()

            # ================= Phase A: vls and uT =================
            with tc.tile_pool(name="pa_w", bufs=1) as pa_w, \
                 tc.tile_pool(name="pa_st", bufs=3) as pa_st:
                v_sb = pa_w.tile([128, DC, D], F32R)   # 32 KB/part
                w_sb = pa_w.tile([128, DC, D], F32R)   # 32 KB/part
                # interleave per-dc chunks so dc=0 deps resolve early
                for dc in range(DC):
                    nc.sync.dma_start(out=v_sb[:, dc], in_=pcv(v, D)[:, dc])
                    nc.sync.dma_start(out=iT_sb[:, dc], in_=pcv(iT, LN)[:, dc])
                    nc.sync.dma_start(out=w_sb[:, dc], in_=pcv(w, D)[:, dc])
                nc.sync.dma_start(out=mlp_sb, in_=pcv(mlpb, D))

                # --- vls[t, e] = sum_d iT[d,t] v[d,e] -> resident bf16 ---
                for tc_ in range(ST):
                    t_sl = slice(tc_ * 128, tc_ * 128 + 128)
                    for eb in range(2):
                        ps = psum_pool.tile([128, 512], F32, name=f"pv{tc_}_{eb}",
                                            tag=f"av{(tc_ * 2 + eb) % 4}")
                        e_sl = slice(eb * 512, eb * 512 + 512)
                        for dc in range(DC):
                            nc.tensor.matmul(
                                ps,
                                iT_sb[:, dc, t_sl],
                                v_sb[:, dc, e_sl],
                                start=(dc == 0), stop=(dc == DC - 1),
                            )
                        nc.vector.tensor_copy(vls_sb[:, tc_, e_sl], ps)

                # --- uT[e, s] = sum_d W[d,e] iT[d,s], split hi/lo -> DRAM ---
                for g in range(G):
                    s_sl = slice(g * 512, g * 512 + 512)
                    for ec in range(DC):
                        ps = psum_pool.tile([128, 512], F32, name=f"pu{g}_{ec}",
                                            tag=f"sc{ec % 4}")
                        e_sl = slice(ec * 128, ec * 128 + 128)
                        for dc in range(DC):
                            nc.tensor.matmul(
                                ps,
                                w_sb[:, dc, e_sl],
                                iT_sb[:, dc, s_sl],
                                start=(dc == 0), stop=(dc == DC - 1),
                            )
                        # hi = r11(u): round-to-nearest, 11 explicit mantissa
                        # bits -- the grid the fp32r DMA write and PE operand
                        # read use; lo = u - hi
                        ust = pa_st.tile([128, 512], F32, name="ust", tag="ust")
                        nc.vector.tensor_copy(ust, ps)
                        uhst = pa_st.tile([128, 512], F32, name="uhst", tag="uhst")
                        nc.vector.tensor_scalar(
                            out=uhst.bitcast(I32), in0=ust.bitcast(I32),
                            scalar1=0x800, scalar2=None,
                            op0=mybir.AluOpType.add,
                        )
                        nc.vector.tensor_scalar(
                            out=uhst.bitcast(I32), in0=uhst.bitcast(I32),
                            scalar1=-4096, scalar2=None,
                            op0=mybir.AluOpType.bitwise_and,
                        )
                        nc.vector.tensor_sub(ust, ust, uhst)
                        nc.gpsimd.dma_start(out=uT_d[g, :, ec, :],
                                            in_=uhst.bitcast(F32R))
                        nc.gpsimd.dma_start(out=uTl_d[g, :, ec, :],
                                            in_=ust.bitcast(F32R))

            # ================= Phase B: attention + MLP =================
            with tc.tile_pool(name="pb_u", bufs=1) as pb_u, \
                 tc.tile_pool(name="pb_att", bufs=1) as pb_att, \
                 tc.tile_pool(name="pb_exp", bufs=2) as pb_exp, \
                 tc.tile_pool(name="pb_ret", bufs=2) as pb_ret, \
                 tc.tile_pool(name="pb_st", bufs=2) as pb_st, \
                 tc.tile_pool(name="pb_io", bufs=2) as pb_io:
                def load_u(g):
                    # chunked per-ec so the first scores matmuls only wait on
                    # the ec=0 chunks; sync queue (no xbar use -> safe)
                    t = pb_u.tile([128, DC, 512], F32R, name="uT_g", tag="uT_g")
                    tl = pb_u.tile([128, DC, 512], F32R, name="uTl_g",
                                   tag="uTl_g")
                    for ec in range(DC):
                        nc.sync.dma_start(out=t[:, ec, :],
                                          in_=uT_d[g, :, ec, :])
                        nc.sync.dma_start(out=tl[:, ec, :],
                                          in_=uTl_d[g, :, ec, :])
                    return t, tl

                u_tiles = {0: load_u(0)}
                for g in range(G):
                    gs = slice(g * 512, g * 512 + 512)
                    uT_g, uTl_g = u_tiles[g]
                    attT = pb_att.tile([128, ST, 512], BF16, name="attT",
                                       tag="attT")
                    ret_t = pb_ret.tile([128, DC, 512], BF16, name="ret",
                                        tag="ret")

                    for st4 in range(4):
                        si = g * 4 + st4
                        u_sl = slice(st4 * 128, st4 * 128 + 128)

                        scs = [
                            psum_pool.tile([128, 512], F32, name=f"sc{si}_{tb}",
                                           tag=f"sc{tb}")
                            for tb in range(TB)
                        ]
                        for ec in range(DC):
                            first = ec == 0
                            last = ec == DC - 1
                            lhs_h = uT_g[:, ec, u_sl]
                            lhs_l = uTl_g[:, ec, u_sl]
                            for tb in range(TB):
                                t_sl = slice(tb * 512, tb * 512 + 512)
                                nc.tensor.matmul(
                                    scs[tb], lhs_h,
                                    iT_sb[:, ec, t_sl],
                                    start=first, stop=False,
                                )
                                nc.tensor.matmul(
                                    scs[tb], lhs_l,
                                    iT_sb[:, ec, t_sl],
                                    start=False, stop=last,
                                )

                        # Per-t-block softmax: local max + exp immediately
                        # (frees each PSUM bank early), then algebraic
                        # rescale by f_tb = e^(m_tb - M) / S.
                        st_t = pb_st.tile([128, 24], F32, name="st_t", tag="stats")
                        negm4 = st_t[:, 0:4]
                        sums = st_t[:, 4:8]
                        negM = st_t[:, 8:9]
                        S = st_t[:, 9:10]
                        recip = st_t[:, 10:11]
                        g4 = st_t[:, 12:16]
                        f4 = st_t[:, 16:20]
                        gs4 = st_t[:, 20:24]
                        exp_t = pb_exp.tile([128, LN], BF16, name="exp_t",
                                            tag="exp", bufs=1)
                        for tb in range(TB):
                            nc.vector.reduce_max(negm4[:, tb:tb + 1], scs[tb],
                                                 axis=Axis.X, negate=True)
                            nc.scalar.activation(
                                out=exp_t[:, tb * 512:tb * 512 + 512], in_=scs[tb],
                                func=Act.Exp, bias=negm4[:, tb:tb + 1], scale=1.0,
                                accum_out=sums[:, tb:tb + 1],
                            )
                        nc.vector.tensor_reduce(negM, negm4, axis=Axis.X,
                                                op=mybir.AluOpType.min)
                        nc.scalar.activation(out=g4, in_=negm4, func=Act.Exp,
                                             bias=negM, scale=-1.0)
                        nc.vector.tensor_mul(gs4, g4, sums)
                        nc.vector.reduce_sum(S, gs4, axis=Axis.X)
                        nc.vector.reciprocal(recip, S)
                        nc.vector.tensor_scalar_mul(f4, g4, recip)

                        att_t = pb_exp.tile([128, LN], BF16, name="att_t",
                                            tag="att")
                        for tb in range(TB):
                            nc.scalar.activation(
                                out=att_t[:, tb * 512:tb * 512 + 512],
                                in_=exp_t[:, tb * 512:tb * 512 + 512],
                                func=Act.Copy, bias=0.0,
                                scale=f4[:, tb:tb + 1],
                            )
                        # DVE 32x32 block transpose: the xbar DMA
                        # transpose corrupts block-max entries whenever other
                        # DMA traffic is in flight (intermittent, known HW
                        # issue) -- do not switch this back to
                        # dma_start_transpose.
                        for pb in range(4):
                            for bi in range(4):
                                nc.vector.transpose(
                                    attT[pb * 32:(pb + 1) * 32, :,
                                         st4 * 128 + bi * 32:
                                         st4 * 128 + bi * 32 + 32],
                                    att_t[bi * 32:(bi + 1) * 32, :].rearrange(
                                        "p (t c x) -> p t c x", c=4, x=32
                                    )[:, :, pb, :],
                                )
                        if dbg:
                            nc.gpsimd.dma_start(out=attd.ap()[si], in_=att_t)

                    if dbg:
                        nc.gpsimd.dma_start(out=attTd.ap()[g], in_=attT)
                    # att @ vls (+ residual i) -> retT[e, s-block], bf16
                    for ec in range(DC):
                        psa = psum_pool.tile([128, 512], F32, name=f"pa{g}_{ec}",
                                             tag=f"av{ec % 4}")
                        e_sl = slice(ec * 128, ec * 128 + 128)
                        for tc_ in range(ST):
                            nc.tensor.matmul(
                                psa,
                                vls_sb[:, tc_, e_sl],
                                attT[:, tc_, :],
                                start=(tc_ == 0), stop=(tc_ == ST - 1),
                            )
                        nc.vector.tensor_add(ret_t[:, ec, :], psa,
                                             iT_sb[:, ec, gs].bitcast(F32))

                    # (ret @ mlp) -> leaky relu -> + bias -> out
                    for st4 in range(4):
                        si = g * 4 + st4
                        s_sl = slice(si * 128, si * 128 + 128)
                        bias_t = pb_io.tile([128, D], BF16, name="bias_t",
                                            tag="bias")
                        nc.gpsimd.dma_start(out=bias_t, in_=bias.ap()[s_sl, :])
                        out_t = pb_io.tile([128, D], F32, name="out_t", tag="out")
                        for ob in range(2):
                            pso = psum_pool.tile([128, 512], F32,
                                                 name=f"po{si}_{ob}",
                                                 tag=f"sc{ob}")
                            o_sl = slice(ob * 512, ob * 512 + 512)
                            for ec in range(DC):
                                nc.tensor.matmul(
                                    pso,
                                    ret_t[:, ec, st4 * 128:st4 * 128 + 128],
                                    mlp_sb[:, ec, o_sl],
                                    start=(ec == 0), stop=(ec == DC - 1),
                                )
                            nc.scalar.activation(
                                out=out_t[:, o_sl], in_=pso, func=Act.Prelu,
                                bias=0.0, scale=1.0, alpha=alpha_ap,
                            )
                        nc.vector.tensor_add(out_t, out_t, bias_t)
                        nc.gpsimd.dma_start(out=out_d.ap()[s_sl, :], in_=out_t)
                    if g + 1 < G:
                        # prefetch next group's u during att@vls + mlp
                        u_tiles[g + 1] = load_u(g + 1)

            _psum_cm.__exit__(None, None, None)

    nc.compile()
    return nc


def _get_nc():
    global _cached_nc
    if _cached_nc is None:
        _cached_nc = _build()
    return _cached_nc


def _prep_in_maps(i, k, q, v, mlp, bias):
    i = np.asarray(i, dtype=np.float32)
    k = np.asarray(k, dtype=np.float32)
    q = np.asarray(q, dtype=np.float32)
    v = np.asarray(v, dtype=np.float32)
    mlp = np.asarray(mlp, dtype=np.float32)
    bias = np.asarray(bias, dtype=np.float32)

    w = (q.astype(np.float64) @ k.astype(np.float64).T).astype(np.float32)
    shared = dict(
        w=w, v=v,
        mlpb=mlp.astype(ml_dtypes.bfloat16),
        bias=bias.astype(ml_dtypes.bfloat16),
    )
    in_maps = []
    for b in range(N_CORES):
        iT = np.ascontiguousarray(i[b].T)
        in_maps.append(dict(iT=iT, **shared))
    return in_maps


def kernel(i, k, q, v, mlp, bias):
    in_maps = _prep_in_maps(i, k, q, v, mlp, bias)
    nc = _get_nc()
    res = bass_utils.run_bass_kernel_spmd(nc, in_maps, core_ids=list(range(N_CORES)))
    return np.stack([res.results[b]["out"] for b in range(N_CORES)])
